# revision 1
# baseline (speedup 1.0000x reference)
"""Trainium2 Bass kernel for nn_BitFeedForward (BitNet b1.58 FFN).

Math (forward values of the reference):
  x_int  = round(x * 127/max|x|_row)            (ints in [-127,127] -> exact in bf16)
  w_tern = clip(round(w / mean|w|), -1, 1)      (ternary -> exact in fp8e4)
  h_int  = x_int @ w_up_tern^T                  (exact integer math in bf16xfp8 matmul, fp32 PSUM)
  g      = relu(h)^2 ; g_int = round(g * 127/max g_row)
  out    = (g_int @ w_down_tern^T) * mean|w_dn| * maxg_row/127

Sharding: pure data-parallel over the 16384 tokens -> 2048 tokens/core, full
weights replicated to all 8 cores, no collectives.

Layout strategy: ALL HBM traffic is natural-layout (large contiguous
descriptors); every transpose happens on-chip via multi-block DMA-xbar
transposes (one instruction per [128, N*128] tile -> [128, N, 128]).
"""
import sys

sys.path.insert(0, "/opt/trn_rl_repo")

import numpy as np
from contextlib import ExitStack

import concourse.bass as bass  # noqa: F401
import concourse.mybir as mybir
import concourse.tile as tile
from concourse import bacc
from concourse.bass_utils import run_bass_kernel_spmd

F32 = mybir.dt.float32
BF16 = mybir.dt.bfloat16
F8 = mybir.dt.float8e4
AX = mybir.AxisListType
OP = mybir.AluOpType
AF = mybir.ActivationFunctionType

N_CORES = 8
B, S, H = 4, 4096, 2048
I = 4096
M_TOT = B * S          # 16384 tokens
M_CORE = M_TOT // N_CORES
P = 128
KUP = H // P           # 16 k-chunks for MM1 (contract over H)
KDN = I // P           # 32 k-chunks for MM2 (contract over I)
C_RND = 12582912.0     # 1.5 * 2**23 : fp32 round-to-nearest-even trick
QB = 127.0
EPS = 1e-5
INV127 = 1.0 / 127.0
WBLK = 1024            # natural weight-load width (f32 elems per partition row)


def build_nc(m_core=M_CORE, parts="all"):
    nblk = m_core // P
    nc = bacc.Bacc("TRN2", target_bir_lowering=False, debug=False)
    x_d = nc.dram_tensor("x", [m_core, H], F32, kind="ExternalInput")
    wup_d = nc.dram_tensor("w_up", [I, H], F32, kind="ExternalInput")
    wdn_d = nc.dram_tensor("w_down", [H, I], F32, kind="ExternalInput")
    out_d = nc.dram_tensor("out", [m_core, H], F32, kind="ExternalOutput")
    x_ap, wup_ap, wdn_ap, out_ap = x_d.ap(), wup_d.ap(), wdn_d.ap(), out_d.ap()

    with tile.TileContext(nc) as tc, ExitStack() as ctx:
        wres = ctx.enter_context(tc.tile_pool(name="wres", bufs=1))
        wstage = ctx.enter_context(tc.tile_pool(name="wstage", bufs=2))
        wqpool = ctx.enter_context(tc.tile_pool(name="wqpool", bufs=2))
        wtpool = ctx.enter_context(tc.tile_pool(name="wtpool", bufs=2))
        xpool = ctx.enter_context(tc.tile_pool(name="xpool", bufs=1))
        xipool = ctx.enter_context(tc.tile_pool(name="xipool", bufs=1))
        xtpool = ctx.enter_context(tc.tile_pool(name="xtpool", bufs=2))
        hpool = ctx.enter_context(tc.tile_pool(name="hpool", bufs=1))
        gtmp = ctx.enter_context(tc.tile_pool(name="gtmp", bufs=1))
        gipool = ctx.enter_context(tc.tile_pool(name="gipool", bufs=3))
        gtpool = ctx.enter_context(tc.tile_pool(name="gtpool", bufs=2))
        opool = ctx.enter_context(tc.tile_pool(name="opool", bufs=2))
        sm = ctx.enter_context(tc.tile_pool(name="sm", bufs=2))
        single = ctx.enter_context(tc.tile_pool(name="single", bufs=1))
        psA = ctx.enter_context(tc.tile_pool(name="psA", bufs=4, space="PSUM"))
        psB = ctx.enter_context(tc.tile_pool(name="psB", bufs=4, space="PSUM"))

        # resident quantized transposed weights (fp8 ternary), K-major
        wupT = wres.tile([P, KUP, I], F8, tag="wupT")    # [k-in-chunk, kc, i]
        wdnT = wres.tile([P, KDN, H], F8, tag="wdnT")    # [k-in-chunk, kc, h]
        ones_sb = single.tile([P, P], F32, tag="ones")
        nc.vector.memset(ones_sb, 1.0)
        cbias = single.tile([P, 1], F32, tag="cbias")
        nc.vector.memset(cbias, C_RND)
        pacc_up = single.tile([P, 64], F32, tag="pacc_up")
        pacc_dn = single.tile([P, 64], F32, tag="pacc_dn")

        # (row0, col0) -> natural [128, WBLK] f32 slice of a weight matrix
        def w_slices(w_ap_, nrows, ncols):
            for rb in range(nrows // P):
                for cb in range(ncols // WBLK):
                    yield rb, cb, w_ap_[rb * P:(rb + 1) * P,
                                        cb * WBLK:(cb + 1) * WBLK]

        def weight_pass_a(w_ap_, nrows, ncols, pacc, label):
            for idx, (rb, cb, src) in enumerate(w_slices(w_ap_, nrows, ncols)):
                stage = wstage.tile([P, WBLK], F32, tag="wstage",
                                    name=f"wsA_{label}_{idx}")
                nc.sync.dma_start(out=stage, in_=src)
                nc.scalar.activation(out=stage, in_=stage, func=AF.Abs,
                                     accum_out=pacc[:, idx:idx + 1])

        def weight_stats(pacc, label):
            sumv = sm.tile([P, 1], F32, tag="wsum", name=f"wsum_{label}")
            nc.vector.tensor_reduce(out=sumv, in_=pacc, axis=AX.X, op=OP.add)
            ps = psA.tile([P, 512], F32, tag="psA", name=f"wps_{label}")
            nc.tensor.matmul(ps[:, 0:1], lhsT=ones_sb, rhs=sumv, start=True, stop=True)
            mean_t = sm.tile([P, 1], F32, tag="wmean", name=f"wmean_{label}")
            nc.vector.tensor_scalar(out=mean_t, in0=ps[:, 0:1], scalar1=1.0 / float(I * H),
                                    scalar2=EPS, op0=OP.mult, op1=OP.max)
            rinv_t = sm.tile([P, 1], F32, tag="wrinv", name=f"wrinv_{label}")
            nc.vector.reciprocal(out=rinv_t, in_=mean_t)
            return mean_t, rinv_t

        def weight_pass_b(w_ap_, nrows, ncols, rinv_t, resident, transposed_cols, label):
            # natural load -> quantize -> one multi-block xbar transpose ->
            # ternary fp8 into resident K-major buffer.
            # resident[p, kc, r] = tern(w[r_glob, kc*128 + p])
            kpb = WBLK // P  # k-chunks covered per natural block (16)
            for idx, (rb, cb, src) in enumerate(w_slices(w_ap_, nrows, ncols)):
                stage = wstage.tile([P, WBLK], F32, tag="wstage",
                                    name=f"wsB_{label}_{idx}")
                nc.sync.dma_start(out=stage, in_=src)
                nc.vector.tensor_scalar(out=stage, in0=stage, scalar1=rinv_t,
                                        scalar2=C_RND, op0=OP.mult, op1=OP.add)
                wq = wqpool.tile([P, WBLK], BF16, tag="wq", name=f"wq_{label}_{idx}")
                nc.vector.tensor_scalar(out=wq, in0=stage, scalar1=C_RND, scalar2=1.0,
                                        op0=OP.subtract, op1=OP.min)
                wqT = wtpool.tile([P, kpb, P], BF16, tag="wqT", name=f"wqT_{label}_{idx}")
                nc.sync.dma_start(out=wqT, in_=wq, transpose=True)
                # global k-chunk range covered by this block's columns
                kc0 = cb * kpb
                nc.gpsimd.tensor_scalar(
                    out=resident[:, kc0:kc0 + kpb, rb * P:(rb + 1) * P],
                    in0=wqT, scalar1=-1.0, scalar2=None, op0=OP.max)

        def x_prep(b):
            x_sb = xpool.tile([P, H], F32, tag="x", name=f"x_{b}")
            nc.sync.dma_start(out=x_sb, in_=x_ap[b * P:(b + 1) * P, :])
            mx = sm.tile([P, 1], F32, tag="mx", name=f"mx_{b}")
            nc.vector.tensor_reduce(out=mx, in_=x_sb, axis=AX.X, op=OP.max,
                                    apply_absolute_value=True)
            mxc = sm.tile([P, 1], F32, tag="mxc", name=f"mxc_{b}")
            nc.vector.tensor_scalar(out=mxc, in0=mx, scalar1=EPS, scalar2=None, op0=OP.max)
            rx = sm.tile([P, 1], F32, tag="rx", name=f"rx_{b}")
            nc.vector.reciprocal(out=rx, in_=mxc)
            sclx = sm.tile([P, 1], F32, tag="sclx", name=f"sclx_{b}")
            nc.vector.tensor_scalar(out=sclx, in0=rx, scalar1=QB, scalar2=None, op0=OP.mult)
            nc.vector.tensor_scalar(out=x_sb, in0=x_sb, scalar1=sclx, scalar2=C_RND,
                                    op0=OP.mult, op1=OP.add)
            x_int = xipool.tile([P, H], BF16, tag="xi", name=f"xi_{b}")
            nc.vector.tensor_scalar(out=x_int, in0=x_sb, scalar1=C_RND, scalar2=None,
                                    op0=OP.subtract)
            x_intT = xtpool.tile([P, KUP, P], BF16, tag="xT", name=f"xT_{b}")
            nc.sync.dma_start(out=x_intT, in_=x_int, transpose=True)
            return mxc, x_intT

        def mm_block(b, mxc, x_intT, mean_up, mean_dn):
            c1 = sm.tile([P, 1], F32, tag="c1", name=f"c1_{b}")
            nc.vector.tensor_scalar(out=c1, in0=mxc, scalar1=mean_up, scalar2=INV127,
                                    op0=OP.mult, op1=OP.mult)
            hpmax = sm.tile([P, 8], F32, tag="hpmax", name=f"hpmax_{b}")
            h_sb = hpool.tile([P, I], F32, tag="h", name=f"h_{b}")
            for ns in range(I // 512):
                ps = psA.tile([P, 512], F32, tag="psA", name=f"ps1_{b}_{ns}")
                for k in range(KUP):
                    nc.tensor.matmul(ps, lhsT=x_intT[:, k, :],
                                     rhs=wupT[:, k, ns * 512:(ns + 1) * 512],
                                     start=(k == 0), stop=(k == KUP - 1))
                nc.vector.tensor_reduce(out=hpmax[:, ns:ns + 1], in_=ps, axis=AX.X, op=OP.max)
                nc.scalar.activation(out=h_sb[:, ns * 512:(ns + 1) * 512], in_=ps, func=AF.Copy)
            hp = sm.tile([P, 1], F32, tag="hp", name=f"hp_{b}")
            nc.vector.tensor_reduce(out=hp, in_=hpmax, axis=AX.X, op=OP.max)
            hr = sm.tile([P, 1], F32, tag="hr", name=f"hr_{b}")
            nc.vector.tensor_scalar(out=hr, in0=hp, scalar1=0.0, scalar2=c1,
                                    op0=OP.max, op1=OP.mult)   # relu(hp)*c1
            gmaxc = sm.tile([P, 1], F32, tag="gmaxc", name=f"gmaxc_{b}")
            nc.vector.tensor_scalar(out=gmaxc, in0=hr, scalar1=hr, scalar2=EPS,
                                    op0=OP.mult, op1=OP.max)   # max(hr^2, EPS)
            rg = sm.tile([P, 1], F32, tag="rg", name=f"rg_{b}")
            nc.vector.reciprocal(out=rg, in_=gmaxc)
            sclg = sm.tile([P, 1], F32, tag="sclg", name=f"sclg_{b}")
            nc.vector.tensor_scalar(out=sclg, in0=rg, scalar1=QB, scalar2=None, op0=OP.mult)
            c1sq = sm.tile([P, 1], F32, tag="c1sq", name=f"c1sq_{b}")
            nc.vector.tensor_scalar(out=c1sq, in0=c1, scalar1=c1, scalar2=None, op0=OP.mult)
            p1sq = sm.tile([P, 1], F32, tag="p1sq", name=f"p1sq_{b}")
            nc.vector.tensor_scalar(out=p1sq, in0=sclg, scalar1=c1sq, scalar2=None, op0=OP.mult)
            corr2 = sm.tile([P, 1], F32, tag="corr2", name=f"corr2_{b}")
            nc.vector.tensor_scalar(out=corr2, in0=gmaxc, scalar1=mean_dn, scalar2=INV127,
                                    op0=OP.mult, op1=OP.mult)

            gintT = gtpool.tile([P, KDN, P], BF16, tag="gT", name=f"gT_{b}")
            for ns in range(I // 512):
                sl = slice(ns * 512, (ns + 1) * 512)
                r2 = gtmp.tile([P, 512], F32, tag="r2", name=f"r2_{b}_{ns}")
                nc.vector.scalar_tensor_tensor(out=r2, in0=h_sb[:, sl], scalar=0.0,
                                               in1=h_sb[:, sl], op0=OP.max, op1=OP.mult)
                nc.scalar.activation(out=r2, in_=r2, func=AF.Identity, bias=cbias, scale=p1sq)
                g_i = gipool.tile([P, 512], BF16, tag="gi", name=f"gi_{b}_{ns}")
                nc.vector.tensor_scalar(out=g_i, in0=r2, scalar1=C_RND, scalar2=None,
                                        op0=OP.subtract)
                nc.sync.dma_start(out=gintT[:, ns * 4:(ns + 1) * 4, :], in_=g_i,
                                  transpose=True)
            for n2 in range(H // 512):
                ps2 = psB.tile([P, 512], F32, tag="psB", name=f"ps2_{b}_{n2}")
                for k in range(KDN):
                    nc.tensor.matmul(ps2, lhsT=gintT[:, k, :],
                                     rhs=wdnT[:, k, n2 * 512:(n2 + 1) * 512],
                                     start=(k == 0), stop=(k == KDN - 1))
                o_sb = opool.tile([P, 512], F32, tag="o", name=f"o_{b}_{n2}")
                nc.scalar.activation(out=o_sb, in_=ps2, func=AF.Copy, scale=corr2)
                nc.sync.dma_start(out=out_ap[b * P:(b + 1) * P, n2 * 512:(n2 + 1) * 512],
                                  in_=o_sb)

        # ---- emission ----
        do_w = parts in ("all", "weights")
        do_b = parts in ("all", "blocks")
        xq = {}
        if do_b:
            for b in range(min(2, nblk)):
                xq[b] = x_prep(b)

        if do_w:
            weight_pass_a(wup_ap, I, H, pacc_up, "up")
            mean_up, rinv_up = weight_stats(pacc_up, "up")
            weight_pass_b(wup_ap, I, H, rinv_up, wupT, KUP, "up")
            weight_pass_a(wdn_ap, H, I, pacc_dn, "dn")
            mean_dn, rinv_dn = weight_stats(pacc_dn, "dn")
            weight_pass_b(wdn_ap, H, I, rinv_dn, wdnT, KDN, "dn")
        else:
            nc.gpsimd.memset(wupT, 0.0)
            nc.gpsimd.memset(wdnT, 0.0)
            nc.vector.memset(pacc_up, 1.0)
            nc.vector.memset(pacc_dn, 1.0)
            mean_up, _ = weight_stats(pacc_up, "up")
            mean_dn, _ = weight_stats(pacc_dn, "dn")

        if do_b:
            for b in range(nblk):
                mxc, x_intT = xq.pop(b)
                mm_block(b, mxc, x_intT, mean_up, mean_dn)
                if b + 2 < nblk:
                    xq[b + 2] = x_prep(b + 2)
        else:
            o_sb = opool.tile([P, 512], F32, tag="o", name="o_dummy")
            nc.vector.memset(o_sb, 0.0)
            nc.sync.dma_start(out=out_ap[0:P, 0:512], in_=o_sb)

    nc.compile()
    return nc


_NC_CACHE = {}


def _get_nc(m_core=M_CORE):
    if m_core not in _NC_CACHE:
        _NC_CACHE[m_core] = build_nc(m_core)
    return _NC_CACHE[m_core]


def kernel(hidden_states, w_up, w_down):
    x = np.ascontiguousarray(np.asarray(hidden_states, dtype=np.float32).reshape(M_TOT, H))
    w_up = np.ascontiguousarray(np.asarray(w_up, dtype=np.float32))
    w_down = np.ascontiguousarray(np.asarray(w_down, dtype=np.float32))
    nc = _get_nc()
    in_maps = [
        {"x": x[i * M_CORE:(i + 1) * M_CORE], "w_up": w_up, "w_down": w_down}
        for i in range(N_CORES)
    ]
    res = run_bass_kernel_spmd(nc, in_maps, list(range(N_CORES)))
    out = np.concatenate([res.results[i]["out"] for i in range(N_CORES)], axis=0)
    return out.reshape(B, S, H).astype(np.float32)



# revision 10
# speedup vs baseline: 1.4348x; 1.4348x over previous
"""Trainium2 Bass kernel for nn_BitFeedForward (BitNet b1.58 FFN).

Math (forward values of the reference):
  x_int  = round(x * 127/max|x|_row)            (ints in [-127,127] -> exact in bf16)
  w_tern = clip(round(w / mean|w|), -1, 1)      (ternary -> exact in fp8e4)
  h_int  = x_int @ w_up_tern^T                  (exact integer math in bf16xfp8 matmul, fp32 PSUM)
  g      = relu(h)^2 ; g_int = round(g * 127/max g_row)
  out    = (g_int @ w_down_tern^T) * mean|w_dn| * maxg_row/127

Sharding: pure data-parallel over the 16384 tokens -> 2048 tokens/core, full
weights replicated to all 8 cores, no collectives.

v2 engine plan (from trace analysis of v1):
  - weight quantize: ACT does scale+round (fp32-internal +C trick), GPSIMD does
    (-C, clip-low) -> bf16, DMA-xbar transpose, DVE does (clip-high) -> fp8
    strided into K-major resident slices.  No slow AP-scalar DVE ops, no
    gpsimd fp8 writes.
  - r2 = relu(h_int)^2 staged bf16 and spilled to DRAM so MM2 can lag MM1 by
    5 blocks (rides out the weight-prep window) without SBUF cost.
  - resident weights are 8 (up) + 4 (dn) independent slice tiles so matmuls
    unlock progressively while quantize streams.
  - MM1/MM2 interleaved emission keeps the PE queue dense.
"""
import sys

sys.path.insert(0, "/opt/trn_rl_repo")

import numpy as np
from contextlib import ExitStack

import concourse.bass as bass  # noqa: F401
import concourse.mybir as mybir
import concourse.tile as tile
from concourse import bacc
from concourse.bass_utils import run_bass_kernel_spmd

F32 = mybir.dt.float32
BF16 = mybir.dt.bfloat16
F8 = mybir.dt.float8e4
AX = mybir.AxisListType
OP = mybir.AluOpType
AF = mybir.ActivationFunctionType

N_CORES = 8
B, S, H = 4, 4096, 2048
I = 4096
M_TOT = B * S          # 16384 tokens
M_CORE = M_TOT // N_CORES
P = 128
KUP = H // P           # 16 k-chunks for MM1 (contract over H)
KDN = I // P           # 32 k-chunks for MM2 (contract over I)
C_RND = 12582912.0     # 1.5 * 2**23 : fp32 round-to-nearest-even trick
QB = 127.0
EPS = 1e-5
INV127 = 1.0 / 127.0
WB = 1024              # natural weight-block width (f32 elems per partition)
KPB = WB // P          # k-chunks per weight block (8)
SKEW = 5               # MM2(b) emitted after MM1(b+SKEW)


def build_nc(m_core=M_CORE):
    nblk = m_core // P
    nc = bacc.Bacc("TRN2", target_bir_lowering=False, debug=False)
    x_d = nc.dram_tensor("x", [m_core, H], F32, kind="ExternalInput")
    wup_d = nc.dram_tensor("w_up", [I, H], F32, kind="ExternalInput")
    wdn_d = nc.dram_tensor("w_down", [H, I], F32, kind="ExternalInput")
    out_d = nc.dram_tensor("out", [m_core, H], F32, kind="ExternalOutput")
    x_ap, wup_ap, wdn_ap, out_ap = x_d.ap(), wup_d.ap(), wdn_d.ap(), out_d.ap()

    with tile.TileContext(nc) as tc, ExitStack() as ctx:
        wres = ctx.enter_context(tc.tile_pool(name="wres", bufs=1))
        wstage = ctx.enter_context(tc.tile_pool(name="wstage", bufs=2))
        wqpool = ctx.enter_context(tc.tile_pool(name="wqpool", bufs=2))
        wtpool = ctx.enter_context(tc.tile_pool(name="wtpool", bufs=2))
        xpool = ctx.enter_context(tc.tile_pool(name="xpool", bufs=1))
        xipool = ctx.enter_context(tc.tile_pool(name="xipool", bufs=1))
        xtpool = ctx.enter_context(tc.tile_pool(name="xtpool", bufs=2))
        r2pool = ctx.enter_context(tc.tile_pool(name="r2pool", bufs=2))
        gtpool = ctx.enter_context(tc.tile_pool(name="gtpool", bufs=1))
        hcpool = ctx.enter_context(tc.tile_pool(name="hcpool", bufs=2))
        gqpool = ctx.enter_context(tc.tile_pool(name="gqpool", bufs=2))
        gipool = ctx.enter_context(tc.tile_pool(name="gipool", bufs=2))
        opool = ctx.enter_context(tc.tile_pool(name="opool", bufs=1))
        sm = ctx.enter_context(tc.tile_pool(name="sm", bufs=8))
        single = ctx.enter_context(tc.tile_pool(name="single", bufs=1))
        psA = ctx.enter_context(tc.tile_pool(name="psA", bufs=4, space="PSUM"))
        psB = ctx.enter_context(tc.tile_pool(name="psB", bufs=4, space="PSUM"))
        dspill = ctx.enter_context(tc.tile_pool(name="dspill", bufs=nblk, space="DRAM"))

        # resident quantized transposed weights (fp8 ternary), K-major slices
        wupT = [wres.tile([P, KUP, 512], F8, tag=f"wupT{j}", name=f"wupT{j}")
                for j in range(8)]
        wdnT = [wres.tile([P, KDN, 512], F8, tag=f"wdnT{j}", name=f"wdnT{j}")
                for j in range(4)]
        ones_sb = single.tile([P, P], F32, tag="ones")
        nc.vector.memset(ones_sb, 1.0)
        pacc_up = single.tile([P, 64], F32, tag="pacc_up")
        pacc_dn = single.tile([P, 64], F32, tag="pacc_dn")

        # ---------- weight prep ----------
        def wa_block(w_ap_, rb, cb, pacc, idx, label):
            """pass A block: load [P, WB] natural, abs-sum into pacc[:, idx]."""
            stage = wstage.tile([P, WB], F32, tag="wstage", name=f"wsA_{label}_{idx}")
            nc.sync.dma_start(out=stage, in_=w_ap_[rb * P:(rb + 1) * P,
                                                   cb * WB:(cb + 1) * WB])
            nc.vector.tensor_reduce(out=pacc[:, idx:idx + 1], in_=stage, axis=AX.X,
                                    op=OP.add, apply_absolute_value=True)

        def weight_stats(pacc, label):
            sumv = sm.tile([P, 1], F32, tag="wsum", name=f"wsum_{label}")
            nc.vector.tensor_reduce(out=sumv, in_=pacc, axis=AX.X, op=OP.add)
            ps = psB.tile([P, 512], F32, tag="psB", name=f"wps_{label}")
            nc.tensor.matmul(ps[:, 0:1], lhsT=ones_sb, rhs=sumv, start=True, stop=True)
            mean_t = sm.tile([P, 1], F32, tag="wmean", name=f"wmean_{label}")
            nc.vector.tensor_scalar(out=mean_t, in0=ps[:, 0:1], scalar1=1.0 / float(I * H),
                                    scalar2=EPS, op0=OP.mult, op1=OP.max)
            rinv_t = sm.tile([P, 1], F32, tag="wrinv", name=f"wrinv_{label}")
            nc.vector.reciprocal(out=rinv_t, in_=mean_t)
            return mean_t, rinv_t

        def wb_block(w_ap_, rb, cb, rinv_t, resident, kc0, col0, idx, label, ring):
            """pass B block: natural load -> ACT scale+round(+C) -> DVE (-C) bf16
            -> xbar transpose -> ACT Sign -> fp8 K-major resident.
            Sign(round(w/s)) == clip(round(w/s), -1, 1) on integers."""
            stage = wstage.tile([P, WB], F32, tag="wstage", name=f"wsB_{label}_{idx}")
            nc.sync.dma_start(out=stage, in_=w_ap_[rb * P:(rb + 1) * P,
                                                   cb * WB:(cb + 1) * WB])
            nc.scalar.activation(out=stage, in_=stage, func=AF.Copy,
                                 scale=rinv_t, bias=C_RND)
            wq = wqpool.tile([P, WB], BF16, tag="wq", name=f"wq_{label}_{idx}")
            nc.vector.tensor_scalar(out=wq, in0=stage, scalar1=C_RND, scalar2=None,
                                    op0=OP.subtract)
            wqT = wtpool.tile([P, KPB, P], BF16, tag="wqT", name=f"wqT_{label}_{idx}")
            ring.dma_start(out=wqT, in_=wq, transpose=True)
            nc.scalar.activation(
                out=resident[:, kc0:kc0 + KPB, col0:col0 + P],
                in_=wqT, func=AF.Sign)

        # ---------- per-block token pipeline ----------
        def x_prep(b):
            xr = xpool.tile([P, H], F32, tag="x", name=f"x_{b}")
            nc.sync.dma_start(out=xr, in_=x_ap[b * P:(b + 1) * P, :])
            mx = sm.tile([P, 1], F32, tag="mx", name=f"mx_{b}")
            nc.vector.tensor_reduce(out=mx, in_=xr, axis=AX.X, op=OP.max,
                                    apply_absolute_value=True)
            mxc = sm.tile([P, 1], F32, tag="mxc", name=f"mxc_{b}")
            nc.vector.tensor_scalar(out=mxc, in0=mx, scalar1=EPS, scalar2=None, op0=OP.max)
            rx = sm.tile([P, 1], F32, tag="rx", name=f"rx_{b}")
            nc.vector.reciprocal(out=rx, in_=mxc)
            sclx = sm.tile([P, 1], F32, tag="sclx", name=f"sclx_{b}")
            nc.vector.tensor_scalar(out=sclx, in0=rx, scalar1=QB, scalar2=None, op0=OP.mult)
            nc.scalar.activation(out=xr, in_=xr, func=AF.Copy, scale=sclx, bias=C_RND)
            xi = xipool.tile([P, H], BF16, tag="xi", name=f"xi_{b}")
            nc.vector.tensor_scalar(out=xi, in0=xr, scalar1=C_RND, scalar2=None,
                                    op0=OP.subtract)
            xiT = xtpool.tile([P, KUP, P], BF16, tag="xT", name=f"xT_{b}")
            nc.scalar.dma_start(out=xiT, in_=xi, transpose=True)
            return mxc, xiT

        state = {}

        def mm1_start(b, mxc, xiT):
            r2w = r2pool.tile([P, I], BF16, tag="r2", name=f"r2w_{b}")
            rmax = sm.tile([P, 8], F32, tag="rmax", name=f"rmax_{b}")
            c1 = sm.tile([P, 1], F32, tag="c1", name=f"c1_{b}")
            nc.vector.tensor_scalar(out=c1, in0=mxc, scalar1=mean_up, scalar2=INV127,
                                    op0=OP.mult, op1=OP.mult)
            c1sq = sm.tile([P, 1], F32, tag="c1sq", name=f"c1sq_{b}")
            nc.vector.tensor_scalar(out=c1sq, in0=c1, scalar1=c1, scalar2=None, op0=OP.mult)
            state[b] = dict(r2w=r2w, rmax=rmax, c1sq=c1sq, xiT=xiT)

        def mm1_tile(b, ns):
            st = state[b]
            ps = psA.tile([P, 512], F32, tag="psA", name=f"ps1_{b}_{ns}")
            for k in range(KUP):
                nc.tensor.matmul(ps, lhsT=st["xiT"][:, k, :],
                                 rhs=wupT[ns][:, k, :],
                                 start=(k == 0), stop=(k == KUP - 1))
            sl = slice(ns * 512, (ns + 1) * 512)
            hc = hcpool.tile([P, 512], F32, tag="hc", name=f"hc_{b}_{ns}")
            nc.scalar.activation(out=hc, in_=ps, func=AF.Copy)
            nc.vector.scalar_tensor_tensor(out=st["r2w"][:, sl], in0=hc, scalar=0.0,
                                           in1=hc, op0=OP.max, op1=OP.mult)
            nc.vector.tensor_reduce(out=st["rmax"][:, ns:ns + 1], in_=st["r2w"][:, sl],
                                    axis=AX.X, op=OP.max)

        def mm1_finish(b):
            st = state[b]
            hp = sm.tile([P, 1], F32, tag="hp", name=f"hp_{b}")
            nc.vector.tensor_reduce(out=hp, in_=st["rmax"], axis=AX.X, op=OP.max)
            gmaxc = sm.tile([P, 1], F32, tag="gmaxc", name=f"gmaxc_{b}")
            nc.vector.tensor_scalar(out=gmaxc, in0=hp, scalar1=st["c1sq"], scalar2=EPS,
                                    op0=OP.mult, op1=OP.max)
            rg = sm.tile([P, 1], F32, tag="rg", name=f"rg_{b}")
            nc.vector.reciprocal(out=rg, in_=gmaxc)
            spill = dspill.tile([P, I], BF16, tag="spill", name=f"spill_{b}")
            nc.sync.dma_start(out=spill, in_=st["r2w"])
            st["gmaxc"] = gmaxc
            st["rg"] = rg
            st["spill"] = spill
            del st["r2w"], st["rmax"], st["xiT"]

        def gq(b):
            st = state[b]
            rho = sm.tile([P, 1], F32, tag="rho", name=f"rho_{b}")
            nc.vector.tensor_scalar(out=rho, in0=st["rg"], scalar1=QB, scalar2=st["c1sq"],
                                    op0=OP.mult, op1=OP.mult)
            corr2 = sm.tile([P, 1], F32, tag="corr2", name=f"corr2_{b}")
            nc.vector.tensor_scalar(out=corr2, in0=st["gmaxc"], scalar1=mean_dn,
                                    scalar2=INV127, op0=OP.mult, op1=OP.mult)
            r2r = r2pool.tile([P, I], BF16, tag="r2", name=f"r2r_{b}")
            nc.sync.dma_start(out=r2r, in_=st["spill"])
            giT = gtpool.tile([P, KDN, P], BF16, tag="gT", name=f"gT_{b}")
            for ns in range(8):
                sl = slice(ns * 512, (ns + 1) * 512)
                tq = gqpool.tile([P, 512], F32, tag="tq", name=f"tq_{b}_{ns}")
                nc.scalar.activation(out=tq, in_=r2r[:, sl], func=AF.Copy,
                                     scale=rho, bias=C_RND)
                gi = gipool.tile([P, 512], BF16, tag="gi", name=f"gi_{b}_{ns}")
                nc.vector.tensor_scalar(out=gi, in0=tq, scalar1=C_RND,
                                        scalar2=None, op0=OP.subtract)
                nc.sync.dma_start(out=giT[:, 4 * ns:4 * ns + 4, :], in_=gi,
                                  transpose=True)
            st["giT"] = giT
            st["corr2"] = corr2

        def mm2(b):
            st = state[b]
            for n2 in range(4):
                ps2 = psB.tile([P, 512], F32, tag="psB", name=f"ps2_{b}_{n2}")
                for k in range(KDN):
                    nc.tensor.matmul(ps2, lhsT=st["giT"][:, k, :],
                                     rhs=wdnT[n2][:, k, :],
                                     start=(k == 0), stop=(k == KDN - 1))
                o_sb = opool.tile([P, 512], F32, tag="o", name=f"o_{b}_{n2}")
                nc.scalar.activation(out=o_sb, in_=ps2, func=AF.Copy, scale=st["corr2"])
                nc.sync.dma_start(out=out_ap[b * P:(b + 1) * P, n2 * 512:(n2 + 1) * 512],
                                  in_=o_sb)
            del state[b]

        def mm1_full(b):
            for ns in range(8):
                mm1_tile(b, ns)
            mm1_finish(b)

        # ---------- emission ----------
        xq = {0: x_prep(0), 1: x_prep(1)}

        # pass A (up): 64 natural blocks [P, 1024]
        for rb in range(I // P):
            for cb in range(H // WB):
                wa_block(wup_ap, rb, cb, pacc_up, 2 * rb + cb, "up")
        mean_up, rinv_up = weight_stats(pacc_up, "up")

        # pass B (up) slice-interleaved with MM1(0): slice j <- rb 4j..4j+3 x cb 0..1
        mm1_start(0, *xq.pop(0))
        for j in range(8):
            for q in range(8):
                rb = 4 * j + q // 2
                cb = q % 2
                idx = 2 * rb + cb
                ring = nc.sync if (idx % 2 == 0) else nc.scalar
                wb_block(wup_ap, rb, cb, rinv_up, wupT[j], KPB * cb,
                         (rb % 4) * P, idx, "up", ring)
            mm1_tile(0, j)
        mm1_finish(0)

        # pass A (dn): 64 natural blocks
        for rb in range(H // P):
            for cb in range(I // WB):
                wa_block(wdn_ap, rb, cb, pacc_dn, 4 * rb + cb, "dn")

        mm1_start(1, *xq.pop(1))
        mm1_full(1)
        xq[2] = x_prep(2)
        mm1_start(2, *xq.pop(2))
        mm1_full(2)
        xq[3] = x_prep(3)
        mean_dn, rinv_dn = weight_stats(pacc_dn, "dn")
        mm1_start(3, *xq.pop(3))
        mm1_full(3)
        xq[4] = x_prep(4)

        # pass B (dn) slice-interleaved with MM1(4), MM1(5).
        # block (rb, cb) -> wdnT[rb//4][:, KPB*cb: , (rb%4)*128: ]
        # j-group covers rb 2j..2j+1 x cb 0..3 (8 blocks).
        mm1_start(4, *xq.pop(4))
        for j in range(8):
            for q in range(8):
                rb = 2 * j + q // 4
                cb = q % 4
                idx = 4 * rb + cb
                ring = nc.sync if (idx % 2 == 0) else nc.scalar
                wb_block(wdn_ap, rb, cb, rinv_dn, wdnT[rb // 4], KPB * cb,
                         (rb % 4) * P, idx, "dn", ring)
            bb, base = (4, 0) if j < 4 else (5, 8)
            mm1_tile(bb, 2 * j - base)
            mm1_tile(bb, 2 * j + 1 - base)
            if j == 3:
                mm1_finish(4)
                xq[5] = x_prep(5)
                mm1_start(5, *xq.pop(5))
        mm1_finish(5)
        xq[6] = x_prep(6)
        xq[7] = x_prep(7)
        gq(0)
        mm2(0)

        # steady state: MM1(b) ; gq/MM2(b-SKEW)
        for b in range(6, nblk):
            mm1_start(b, *xq.pop(b))
            mm1_full(b)
            if b + 2 < nblk:
                xq[b + 2] = x_prep(b + 2)
            gq(b - SKEW)
            mm2(b - SKEW)
        for b in range(nblk - SKEW, nblk):
            gq(b)
            mm2(b)

    nc.compile()
    return nc


_NC_CACHE = {}


def _get_nc(m_core=M_CORE):
    if m_core not in _NC_CACHE:
        _NC_CACHE[m_core] = build_nc(m_core)
    return _NC_CACHE[m_core]


def kernel(hidden_states, w_up, w_down):
    x = np.ascontiguousarray(np.asarray(hidden_states, dtype=np.float32).reshape(M_TOT, H))
    w_up = np.ascontiguousarray(np.asarray(w_up, dtype=np.float32))
    w_down = np.ascontiguousarray(np.asarray(w_down, dtype=np.float32))
    nc = _get_nc()
    in_maps = [
        {"x": x[i * M_CORE:(i + 1) * M_CORE], "w_up": w_up, "w_down": w_down}
        for i in range(N_CORES)
    ]
    res = run_bass_kernel_spmd(nc, in_maps, list(range(N_CORES)))
    out = np.concatenate([res.results[i]["out"] for i in range(N_CORES)], axis=0)
    return out.reshape(B, S, H).astype(np.float32)


# revision 16
# speedup vs baseline: 1.5829x; 1.1032x over previous
"""Trainium2 Bass kernel for nn_BitFeedForward (BitNet b1.58 FFN).

Math (forward values of the reference):
  x_int  = round(x * 127/max|x|_row)            (ints in [-127,127] -> exact in bf16)
  w_tern = clip(round(w / mean|w|), -1, 1)      (ternary -> exact in fp8e4)
  h_int  = x_int @ w_up_tern^T                  (exact integer math in bf16xfp8 matmul, fp32 PSUM)
  g      = relu(h)^2 ; g_int = round(g * 127/max g_row)
  out    = (g_int @ w_down_tern^T) * mean|w_dn| * maxg_row/127

Sharding: pure data-parallel over the 16384 tokens -> 2048 tokens/core, full
weights replicated to all 8 cores, no collectives.

v4 design (from v2/v3 hardware traces):
  - Weight quantize pipeline is stage-skew emitted (load / ACT scale+round /
    DVE clip(2 ops) / xbar transpose / gpsimd DMA-cast to fp8) so no engine
    queue head-blocks on a full chain; every fp8 conversion rides the DMA
    datapath (free) instead of the 15ns/elem compute-engine fp8 path.
  - h is staged f32 and spilled to DRAM (exact g_int, rel err ~1e-4), MM2
    lags MM1 by 5 blocks to ride out the 134MB weight double-read window.
  - resident ternary weights are 8+4 K-major fp8 slice tiles, unlocked
    progressively while pass B streams.
"""
import sys

sys.path.insert(0, "/opt/trn_rl_repo")

import numpy as np
from contextlib import ExitStack

import concourse.bass as bass  # noqa: F401
import concourse.mybir as mybir
import concourse.tile as tile
from concourse import bacc
from concourse.bass_utils import run_bass_kernel_spmd

F32 = mybir.dt.float32
BF16 = mybir.dt.bfloat16
F8 = mybir.dt.float8e4
AX = mybir.AxisListType
OP = mybir.AluOpType
AF = mybir.ActivationFunctionType

N_CORES = 8
B, S, H = 4, 4096, 2048
I = 4096
M_TOT = B * S          # 16384 tokens
M_CORE = M_TOT // N_CORES
P = 128
KUP = H // P           # 16 k-chunks for MM1 (contract over H)
KDN = I // P           # 32 k-chunks for MM2 (contract over I)
C_RND = 12582912.0     # 1.5 * 2**23 : fp32 round-to-nearest-even trick
QB = 127.0
EPS = 1e-5
INV127 = 1.0 / 127.0
WA = 1024              # pass-A natural block width
WB = 1024              # pass-B natural block width
KPB = WB // P          # k-chunks per B block (8)
SKEW = 5               # MM2(b) emitted after MM1(b+SKEW)


def build_nc(m_core=M_CORE):
    nblk = m_core // P
    nc = bacc.Bacc("TRN2", target_bir_lowering=False, debug=False)
    x_d = nc.dram_tensor("x", [m_core, H], F32, kind="ExternalInput")
    wup_d = nc.dram_tensor("w_up", [I, H], F32, kind="ExternalInput")
    wdn_d = nc.dram_tensor("w_down", [H, I], F32, kind="ExternalInput")
    out_d = nc.dram_tensor("out", [m_core, H], F32, kind="ExternalOutput")
    x_ap, wup_ap, wdn_ap, out_ap = x_d.ap(), wup_d.ap(), wdn_d.ap(), out_d.ap()

    with tile.TileContext(nc) as tc, ExitStack() as ctx:
        wres = ctx.enter_context(tc.tile_pool(name="wres", bufs=1))
        wstage = ctx.enter_context(tc.tile_pool(name="wstage", bufs=3))
        wstageA = ctx.enter_context(tc.tile_pool(name="wstageA", bufs=2))
        wqpool = ctx.enter_context(tc.tile_pool(name="wqpool", bufs=2))
        wtpool = ctx.enter_context(tc.tile_pool(name="wtpool", bufs=2))
        xpool = ctx.enter_context(tc.tile_pool(name="xpool", bufs=1))
        xipool = ctx.enter_context(tc.tile_pool(name="xipool", bufs=1))
        xtpool = ctx.enter_context(tc.tile_pool(name="xtpool", bufs=2))
        hcpool = ctx.enter_context(tc.tile_pool(name="hcpool", bufs=2))
        hrpool = ctx.enter_context(tc.tile_pool(name="hrpool", bufs=1))
        r2pool = ctx.enter_context(tc.tile_pool(name="r2pool", bufs=1))
        gipool = ctx.enter_context(tc.tile_pool(name="gipool", bufs=1))
        gtpool = ctx.enter_context(tc.tile_pool(name="gtpool", bufs=1))
        opool = ctx.enter_context(tc.tile_pool(name="opool", bufs=1))
        sm = ctx.enter_context(tc.tile_pool(name="sm", bufs=8))
        sm3 = ctx.enter_context(tc.tile_pool(name="sm3", bufs=3))
        single = ctx.enter_context(tc.tile_pool(name="single", bufs=1))
        psA = ctx.enter_context(tc.tile_pool(name="psA", bufs=4, space="PSUM"))
        psB = ctx.enter_context(tc.tile_pool(name="psB", bufs=4, space="PSUM"))
        dspill = ctx.enter_context(tc.tile_pool(name="dspill", bufs=nblk, space="DRAM"))

        wupT = [wres.tile([P, KUP, 512], F8, tag=f"wupT{j}", name=f"wupT{j}")
                for j in range(8)]
        wdnT = [wres.tile([P, KDN, 512], F8, tag=f"wdnT{j}", name=f"wdnT{j}")
                for j in range(4)]
        ones_sb = single.tile([P, P], F32, tag="ones")
        nc.vector.memset(ones_sb, 1.0)
        pacc_up = single.tile([P, 64], F32, tag="pacc_up")
        pacc_dn = single.tile([P, 64], F32, tag="pacc_dn")

        # ---------- weight prep ----------
        def wa_block(w_ap_, rb, cb, pacc, idx, label):
            stage = wstageA.tile([P, WA], F32, tag="wstageA", name=f"wsA_{label}_{idx}")
            nc.scalar.dma_start(out=stage, in_=w_ap_[rb * P:(rb + 1) * P,
                                                     cb * WA:(cb + 1) * WA])
            nc.vector.tensor_reduce(out=pacc[:, idx:idx + 1], in_=stage, axis=AX.X,
                                    op=OP.add, apply_absolute_value=True)

        def weight_stats(pacc, label):
            sumv = sm3.tile([P, 1], F32, tag="wsum", name=f"wsum_{label}")
            nc.vector.tensor_reduce(out=sumv, in_=pacc, axis=AX.X, op=OP.add)
            ps = psB.tile([P, 512], F32, tag="psB", name=f"wps_{label}")
            nc.tensor.matmul(ps[:, 0:1], lhsT=ones_sb, rhs=sumv, start=True, stop=True)
            mean_t = sm3.tile([P, 1], F32, tag="wmean", name=f"wmean_{label}")
            nc.vector.tensor_scalar(out=mean_t, in0=ps[:, 0:1], scalar1=1.0 / float(I * H),
                                    scalar2=EPS, op0=OP.mult, op1=OP.max)
            rinv_t = sm3.tile([P, 1], F32, tag="wrinv", name=f"wrinv_{label}")
            nc.vector.reciprocal(out=rinv_t, in_=mean_t)
            return mean_t, rinv_t

        class WBPipe:
            """Stage-skewed pass-B pipeline: each call to step(i) emits
            load(i), p1(i-1), p2(i-2), transpose(i-3), cast(i-4)."""

            def __init__(self, w_ap_, rinv_t, label, blocks):
                # blocks: list of (rb, cb, resident, kc0, col0)
                self.w = w_ap_
                self.rinv = rinv_t
                self.label = label
                self.blocks = blocks
                self.st = {}
                self.n = len(blocks)

            def step(self, i):
                lb = self.label
                if i < self.n:
                    rb, cb, _, _, _ = self.blocks[i]
                    stage = wstage.tile([P, WB], F32, tag="wstage",
                                        name=f"wsB_{lb}_{i}")
                    nc.sync.dma_start(out=stage, in_=self.w[rb * P:(rb + 1) * P,
                                                           cb * WB:(cb + 1) * WB])
                    self.st[i] = stage
                j = i - 1
                if 0 <= j < self.n:
                    nc.scalar.activation(out=self.st[j], in_=self.st[j], func=AF.Copy,
                                         scale=self.rinv, bias=C_RND)
                j = i - 2
                if 0 <= j < self.n:
                    wq = wqpool.tile([P, WB], BF16, tag="wq", name=f"wq_{lb}_{j}")
                    nc.vector.tensor_scalar(out=wq, in0=self.st.pop(j), scalar1=C_RND,
                                            scalar2=None, op0=OP.subtract)
                    self.st[("q", j)] = wq
                j = i - 3
                if 0 <= j < self.n:
                    wqT = wtpool.tile([P, KPB, P], BF16, tag="wqT",
                                      name=f"wqT_{lb}_{j}")
                    nc.sync.dma_start(out=wqT, in_=self.st.pop(("q", j)),
                                      transpose=True)
                    self.st[("t", j)] = wqT
                j = i - 4
                if 0 <= j < self.n:
                    _, _, resident, kc0, col0 = self.blocks[j]
                    nc.scalar.activation(
                        out=resident[:, kc0:kc0 + KPB, col0:col0 + P],
                        in_=self.st.pop(("t", j)), func=AF.Sign)

        # ---------- per-block token pipeline ----------
        def x_prep(b):
            xr = xpool.tile([P, H], F32, tag="x", name=f"x_{b}")
            nc.sync.dma_start(out=xr, in_=x_ap[b * P:(b + 1) * P, :])
            mx = sm3.tile([P, 1], F32, tag="mx", name=f"mx_{b}")
            nc.vector.tensor_reduce(out=mx, in_=xr, axis=AX.X, op=OP.max,
                                    apply_absolute_value=True)
            mxc = sm.tile([P, 1], F32, tag="mxc", name=f"mxc_{b}")
            nc.vector.tensor_scalar(out=mxc, in0=mx, scalar1=EPS, scalar2=None, op0=OP.max)
            rx = sm3.tile([P, 1], F32, tag="rx", name=f"rx_{b}")
            nc.vector.reciprocal(out=rx, in_=mxc)
            sclx = sm3.tile([P, 1], F32, tag="sclx", name=f"sclx_{b}")
            nc.vector.tensor_scalar(out=sclx, in0=rx, scalar1=QB, scalar2=None, op0=OP.mult)
            nc.scalar.activation(out=xr, in_=xr, func=AF.Copy, scale=sclx, bias=C_RND)
            xi = xipool.tile([P, H], BF16, tag="xi", name=f"xi_{b}")
            nc.vector.tensor_scalar(out=xi, in0=xr, scalar1=C_RND, scalar2=None,
                                    op0=OP.subtract)
            xiT = xtpool.tile([P, KUP, P], BF16, tag="xT", name=f"xT_{b}")
            nc.scalar.dma_start(out=xiT, in_=xi, transpose=True)
            return mxc, xiT

        state = {}

        def mm1_start(b, mxc, xiT):
            rmax = sm3.tile([P, 4], F32, tag="rmax", name=f"rmax_{b}")
            c1 = sm.tile([P, 1], F32, tag="c1", name=f"c1_{b}")
            nc.vector.tensor_scalar(out=c1, in0=mxc, scalar1=mean_up, scalar2=INV127,
                                    op0=OP.mult, op1=OP.mult)
            c1sq = sm.tile([P, 1], F32, tag="c1sq", name=f"c1sq_{b}")
            nc.vector.tensor_scalar(out=c1sq, in0=c1, scalar1=c1, scalar2=None, op0=OP.mult)
            spill = dspill.tile([P, I], F32, tag="spill", name=f"spill_{b}")
            state[b] = dict(rmax=rmax, c1sq=c1sq, xiT=xiT, spill=spill, hc=None)

        def mm1_tile(b, ns):
            st = state[b]
            ps = psA.tile([P, 512], F32, tag="psA", name=f"ps1_{b}_{ns}")
            for k in range(KUP):
                nc.tensor.matmul(ps, lhsT=st["xiT"][:, k, :],
                                 rhs=wupT[ns][:, k, :],
                                 start=(k == 0), stop=(k == KUP - 1))
            half = ns % 2
            if half == 0:
                st["hc"] = hcpool.tile([P, 1024], F32, tag="hc",
                                       name=f"hc_{b}_{ns // 2}")
            nc.vector.tensor_scalar(out=st["hc"][:, half * 512:(half + 1) * 512],
                                    in0=ps, scalar1=0.0, scalar2=None, op0=OP.add)
            if half == 1:
                pr = ns // 2
                nc.vector.tensor_reduce(out=st["rmax"][:, pr:pr + 1], in_=st["hc"],
                                        axis=AX.X, op=OP.max)
                nc.gpsimd.dma_start(out=st["spill"][:, pr * 1024:(pr + 1) * 1024],
                                      in_=st["hc"])

        def mm1_finish(b):
            st = state[b]
            hp = sm3.tile([P, 1], F32, tag="hp", name=f"hp_{b}")
            nc.vector.tensor_reduce(out=hp, in_=st["rmax"], axis=AX.X, op=OP.max)
            hpr = sm3.tile([P, 1], F32, tag="hpr", name=f"hpr_{b}")
            nc.vector.tensor_scalar(out=hpr, in0=hp, scalar1=0.0, scalar2=None, op0=OP.max)
            gmaxc = sm3.tile([P, 1], F32, tag="gmaxc", name=f"gmaxc_{b}")
            nc.vector.tensor_scalar(out=gmaxc, in0=hpr, scalar1=hpr, scalar2=st["c1sq"],
                                    op0=OP.mult, op1=OP.mult)
            gmc = sm.tile([P, 1], F32, tag="gmc", name=f"gmc_{b}")
            nc.vector.tensor_scalar(out=gmc, in0=gmaxc, scalar1=EPS, scalar2=None,
                                    op0=OP.max)
            rg = sm.tile([P, 1], F32, tag="rg", name=f"rg_{b}")
            nc.vector.reciprocal(out=rg, in_=gmc)
            st["gmc"] = gmc
            st["rg"] = rg
            del st["rmax"], st["xiT"], st["hc"]

        def gq(b):
            st = state[b]
            rho = sm3.tile([P, 1], F32, tag="rho", name=f"rho_{b}")
            nc.vector.tensor_scalar(out=rho, in0=st["rg"], scalar1=QB, scalar2=st["c1sq"],
                                    op0=OP.mult, op1=OP.mult)
            corr2 = sm3.tile([P, 1], F32, tag="corr2", name=f"corr2_{b}")
            nc.vector.tensor_scalar(out=corr2, in0=st["gmc"], scalar1=mean_dn,
                                    scalar2=INV127, op0=OP.mult, op1=OP.mult)
            giT = gtpool.tile([P, KDN, P], BF16, tag="gT", name=f"gT_{b}")
            for pr in range(4):
                hr = hrpool.tile([P, 1024], F32, tag="hr", name=f"hr_{b}_{pr}")
                nc.gpsimd.dma_start(out=hr,
                                    in_=st["spill"][:, pr * 1024:(pr + 1) * 1024])
                r2c = r2pool.tile([P, 1024], F32, tag="r2c", name=f"r2c_{b}_{pr}")
                nc.vector.scalar_tensor_tensor(out=r2c, in0=hr, scalar=0.0,
                                               in1=hr, op0=OP.max, op1=OP.mult)
                nc.scalar.activation(out=r2c, in_=r2c, func=AF.Copy,
                                     scale=rho, bias=C_RND)
                gi = gipool.tile([P, 1024], BF16, tag="gi", name=f"gi_{b}_{pr}")
                nc.vector.tensor_scalar(out=gi, in0=r2c, scalar1=C_RND,
                                        scalar2=None, op0=OP.subtract)
                nc.sync.dma_start(out=giT[:, 8 * pr:8 * pr + 8, :], in_=gi,
                                  transpose=True)
            st["giT"] = giT
            st["corr2"] = corr2

        def mm2(b):
            st = state[b]
            for n2 in range(4):
                ps2 = psB.tile([P, 512], F32, tag="psB", name=f"ps2_{b}_{n2}")
                for k in range(KDN):
                    nc.tensor.matmul(ps2, lhsT=st["giT"][:, k, :],
                                     rhs=wdnT[n2][:, k, :],
                                     start=(k == 0), stop=(k == KDN - 1))
                o_sb = opool.tile([P, 512], F32, tag="o", name=f"o_{b}_{n2}")
                nc.scalar.activation(out=o_sb, in_=ps2, func=AF.Copy, scale=st["corr2"])
                nc.sync.dma_start(out=out_ap[b * P:(b + 1) * P,
                                               n2 * 512:(n2 + 1) * 512],
                                  in_=o_sb)
            del state[b]

        def mm1_full(b):
            for ns in range(8):
                mm1_tile(b, ns)
            mm1_finish(b)

        # ---------- emission ----------
        xq = {0: x_prep(0), 1: x_prep(1)}

        # pass A (up): 32 blocks [P, 2048]
        for rb in range(I // P):
            for cb in range(H // WA):
                wa_block(wup_ap, rb, cb, pacc_up, 2 * rb + cb, "up")
        mean_up, rinv_up = weight_stats(pacc_up, "up")

        # pass B (up): 64 blocks [P, 1024]; slice j <- rb 4j..4j+3 x cb 0..1
        up_blocks = []
        for j in range(8):
            for q in range(8):
                rb = 4 * j + q // 2
                cb = q % 2
                up_blocks.append((rb, cb, wupT[j], KPB * cb, (rb % 4) * P))
        pipe_up = WBPipe(wup_ap, rinv_up, "up", up_blocks)

        mm1_start(0, *xq.pop(0))
        for i in range(68):
            pipe_up.step(i)
            if i >= 11 and (i - 11) % 8 == 0 and (i - 11) // 8 < 8:
                mm1_tile(0, (i - 11) // 8)
        mm1_finish(0)

        # pass A (dn): 32 blocks [P, 2048]
        for rb in range(H // P):
            for cb in range(I // WA):
                wa_block(wdn_ap, rb, cb, pacc_dn, 4 * rb + cb, "dn")

        mm1_start(1, *xq.pop(1))
        mm1_full(1)
        xq[2] = x_prep(2)
        mm1_start(2, *xq.pop(2))
        mm1_full(2)
        xq[3] = x_prep(3)
        mean_dn, rinv_dn = weight_stats(pacc_dn, "dn")
        mm1_start(3, *xq.pop(3))
        mm1_full(3)
        xq[4] = x_prep(4)

        # pass B (dn): 64 blocks; n2-slice j-group covers rb 2j..2j+1 x cb 0..3
        dn_blocks = []
        for j in range(8):
            for q in range(8):
                rb = 2 * j + q // 4
                cb = q % 4
                dn_blocks.append((rb, cb, wdnT[rb // 4], KPB * cb, (rb % 4) * P))
        pipe_dn = WBPipe(wdn_ap, rinv_dn, "dn", dn_blocks)

        mm1_start(4, *xq.pop(4))
        for i in range(72):
            pipe_dn.step(i)
            if i >= 11 and (i - 11) % 4 == 0:
                t = (i - 11) // 4
                if t < 16:
                    bb, ns = (4, t) if t < 8 else (5, t - 8)
                    if bb == 5 and ns == 0:
                        mm1_finish(4)
                        xq[5] = x_prep(5)
                        mm1_start(5, *xq.pop(5))
                    mm1_tile(bb, ns)
        mm1_finish(5)
        xq[6] = x_prep(6)
        xq[7] = x_prep(7)
        gq(0)
        mm2(0)

        # steady state
        for b in range(6, nblk):
            mm1_start(b, *xq.pop(b))
            mm1_full(b)
            if b + 2 < nblk:
                xq[b + 2] = x_prep(b + 2)
            gq(b - SKEW)
            mm2(b - SKEW)
        for b in range(nblk - SKEW, nblk):
            gq(b)
            mm2(b)

    nc.compile()
    return nc


_NC_CACHE = {}


def _get_nc(m_core=M_CORE):
    if m_core not in _NC_CACHE:
        _NC_CACHE[m_core] = build_nc(m_core)
    return _NC_CACHE[m_core]


def kernel(hidden_states, w_up, w_down):
    x = np.ascontiguousarray(np.asarray(hidden_states, dtype=np.float32).reshape(M_TOT, H))
    w_up = np.ascontiguousarray(np.asarray(w_up, dtype=np.float32))
    w_down = np.ascontiguousarray(np.asarray(w_down, dtype=np.float32))
    nc = _get_nc()
    in_maps = [
        {"x": x[i * M_CORE:(i + 1) * M_CORE], "w_up": w_up, "w_down": w_down}
        for i in range(N_CORES)
    ]
    res = run_bass_kernel_spmd(nc, in_maps, list(range(N_CORES)))
    out = np.concatenate([res.results[i]["out"] for i in range(N_CORES)], axis=0)
    return out.reshape(B, S, H).astype(np.float32)
